# revision 56
# baseline (speedup 1.0000x reference)
"""Luong 'general' attention kernel for Trainium2 (Bass/Tile), 8-core SPMD.

Math (per batch b):
    v_b        = Wa @ dec_ht[b]                      # (H,)
    raw[t]     = enc_hs[b,t,:] . v_b                 # (T,)
    score[t]   = raw[t] + (mask? 0 : -1e4)           # -1e4 << any real score gap
    attn       = softmax(score)                      # masked lanes underflow to 0
    context[b] = sum_t attn[t] * enc_hs[b,t,:]       # masked lanes contribute 0

Sharding: data-parallel over batch B=32 across 8 cores (4 batches/core).

The whole device-side data path runs in fp16 (inputs cast host-side):
halves HBM traffic vs fp32 (the roofline), gives the DVE 2x mode for the
score products, and the PE 1-cycle/row matmul for the context.  Measured
rel err vs the fp32 reference: ~1.0e-2 (dominated by one batch with a
0.79 top-2 score gap; softmax weights there shift by ~1%%).

Engine plan per core (64 score tiles of [128,1024]):
  - DVE : score products (fp16 tensor_mul, 2x mode) for 56 tiles +
          fused mul+reduce (tensor_tensor_reduce) for 8 tiles
  - ACT : free-dim reduce via activation(Copy, accum_out) for 32 tiles,
          exp, PSUM evacuations
  - POOL: free-dim reduce (tensor_reduce) for 24 tiles, v broadcast
  - PE  : v matmuls (Wa^T pre-transposed on host), softmax max/bcast
          helpers, fp16 context matmuls (PSUM accumulation)
  - DMA : WaT (2 MiB) + enc (16 MiB) fp16 -> ~52 us roofline/core
"""

import os
import sys
from contextlib import ExitStack

for _p in ("/root/.axon_site", "/root/.axon_site/_ro/trn_rl_repo",
           "/root/.axon_site/_ro/pypackages", "/opt/trn_rl_repo"):
    if os.path.isdir(_p) and _p not in sys.path:
        sys.path.append(_p)

import numpy as np

import concourse.bass as bass
import concourse.tile as tile
from concourse import bacc, bass_isa, masks, mybir

B, T, H, U = 32, 2048, 1024, 1024
N_CORES = 8
B_LOC = B // N_CORES          # 4 batches per core
TCH = T // 128                # 16 t-chunks of 128 per batch
NSUB = 4                      # enc DMA granularity: 4 chunks of 512 rows
MASK_BIAS = -1.0e4
F32 = mybir.dt.float32
F16 = mybir.dt.float16

# per-batch score routing (index = t-chunk k), balanced to measured HW
# rates (DVE mul 602 / Pool mul 2693 / ACT Identity-accum 1675 /
# DVE reduce 1730 ns per [128,1024] f16 tile):
#   'a' = DVE mul + ACT accum    (6)
#   'p' = POOL mul + ACT accum   (4)
#   'd' = DVE mul + DVE reduce   (4)
#   'q' = POOL mul + DVE reduce  (2)
ROUTE = ['a', 'p', 'd', 'a', 'q', 'a', 'p', 'd',
         'a', 'p', 'd', 'a', 'q', 'd', 'p', 'a']


def emit_kernel(tc, enc, dec_t, mbias, wat, out):
    """Per-core program.  enc:[B_LOC,T,H]f16  dec_t:[128,8,B_LOC]f16
    mbias:[128,B_LOC,TCH]f32  wat:[U,H]f16 (= Wa^T)  out:[B_LOC,H]f32."""
    nc = tc.nc
    with ExitStack() as ctx:
        const_pool = ctx.enter_context(tc.tile_pool(name="const", bufs=1))
        ident = const_pool.tile([128, 128], F32, tag="ident")
        masks.make_identity(nc, ident[:])
        ones_col = const_pool.tile([128, 1], F32, tag="ones_col")
        nc.vector.memset(ones_col[:], 1.0)
        neg_row = const_pool.tile([1, 128], F32, tag="neg_row")
        nc.vector.memset(neg_row[:], -1.0)

        # PE warmer: free-running dummy matmuls that hold the tensor
        # engine's clock/pipeline up through its idle windows (cold-start
        # matmuls measured ~2x slower than warm ones)
        wl = const_pool.tile([128, 1], F16, tag="wl")
        nc.vector.memset(wl[:], 0.0)
        wr = const_pool.tile([128, 256], F16, tag="wr")
        nc.vector.memset(wr[:], 0.0)
        psum_warm = ctx.enter_context(
            tc.tile_pool(name="psum_warm", bufs=2, space="PSUM"))

        def warm_pe(n):
            for _ in range(n):
                wp = psum_warm.tile([1, 256], F32, name="wp", tag="wp",
                                    bufs=2)
                nc.tensor.matmul(wp[:], wl[:], wr[:], start=True, stop=True)

        vrep_pool = ctx.enter_context(tc.tile_pool(name="vrep", bufs=1))
        enc_pool = ctx.enter_context(tc.tile_pool(name="enc", bufs=16))
        scr_pool = ctx.enter_context(tc.tile_pool(name="scr", bufs=3))
        small_pool = ctx.enter_context(tc.tile_pool(name="small", bufs=2))

        # ---------- Phase V: vrep[b] = broadcast(Wa @ dec[b]) ----------
        vreps = []
        with ExitStack() as vctx:
            wat_pool = vctx.enter_context(tc.tile_pool(name="wat", bufs=1))
            vsb_pool = vctx.enter_context(tc.tile_pool(name="vsb", bufs=1))
            psum_v = vctx.enter_context(
                tc.tile_pool(name="psum_v", bufs=1, space="PSUM"))

            dT = vsb_pool.tile([128, 8, B_LOC], F16, tag="dT")
            nc.sync.dma_start(dT[:], dec_t)
            # per-chunk WaT loads so matmul c starts as chunk c lands
            waT = wat_pool.tile([128, 8, H], F16, tag="waT")
            for c in range(8):
                trig = nc.sync if c % 2 == 0 else nc.scalar
                trig.dma_start(waT[:, c, :], wat[c * 128:(c + 1) * 128, :])
            warm_pe(16)

            # batch-0 fast path: its own [1,H] accumulation lands on
            # partition 0 directly (no row-extract DMA), so vrep_0 — which
            # gates the whole score pipeline — is ready ~9 us earlier.
            vps0 = psum_v.tile([1, H], F32, tag="vps0")
            for c in range(8):
                for hh in range(2):
                    nc.tensor.matmul(vps0[:, hh * 512:(hh + 1) * 512],
                                     dT[:, c, 0:1],
                                     waT[:, c, hh * 512:(hh + 1) * 512],
                                     start=(c == 0), stop=(c == 7))
            vb0 = vsb_pool.tile([1, H], F16, tag="vb_0f", name="vb0")
            nc.scalar.copy(vb0[:], vps0[:])
            vrep0 = vrep_pool.tile([128, 1, H], F16, name="vrep_0",
                                   tag="vrep_0")
            nc.gpsimd.partition_broadcast(vrep0[:, 0, :], vb0[:])
            vreps.append(vrep0)

            # vT[b, h] accumulated over u-chunks c (512-wide matmul limit)
            vT_ps = psum_v.tile([B_LOC, H], F32, tag="vT_ps")
            for c in range(8):
                for hh in range(2):
                    nc.tensor.matmul(vT_ps[:, hh * 512:(hh + 1) * 512],
                                     dT[:, c, :],
                                     waT[:, c, hh * 512:(hh + 1) * 512],
                                     start=(c == 0), stop=(c == 7))
            vT_sb = vsb_pool.tile([B_LOC, H], F16, tag="vT_sb")
            nc.scalar.copy(vT_sb[:], vT_ps[:])

            for b in range(1, B_LOC):
                # row b -> partition 0, then broadcast to all 128 partitions
                vb_sb = vsb_pool.tile([1, H], F16, name=f"vb_{b}",
                                      tag=f"vb_{b}")
                nc.scalar.dma_start(vb_sb[:], vT_sb[b:b + 1, :])
                vrep = vrep_pool.tile([128, 1, H], F16, name=f"vrep_{b}",
                                      tag=f"vrep_{b}")
                nc.gpsimd.partition_broadcast(vrep[:, 0, :], vb_sb[:])
                vreps.append(vrep)

        # mask bias for all local batches, [p, b, k]
        mb = const_pool.tile([128, B_LOC, TCH], F32, tag="mb")
        nc.sync.dma_start(mb[:], mbias)

        psum_small = ctx.enter_context(
            tc.tile_pool(name="psum_small", bufs=2, space="PSUM"))
        psum_ctx = ctx.enter_context(
            tc.tile_pool(name="psum_ctx", bufs=2, space="PSUM"))

        # ---------- enc streaming: all 16 sub-chunk DMAs up-front ----------
        # All on the SP (sync) HWDGE queue: it runs no compute, so nothing
        # compute-dependent ever sits at the head of the FIFO; 16 bufs hold
        # every tile so there are no rotation waits either.
        subs_all = []
        for b in range(B_LOC):
            subs = []
            for s in range(NSUB):
                et = enc_pool.tile([128, NSUB, H], F16, name="enc_t",
                                   tag="enc_t", bufs=16)
                nc.sync.dma_start(
                    et[:], enc[b, s * 512:(s + 1) * 512, :]
                    .rearrange("(j p) h -> p j h", p=128))
                subs.append(et)
            subs_all.append(subs)

        # ---------- Main per-batch pipeline ----------
        # Stage A (scores) and stage B (softmax+context) are emitted
        # software-pipelined: B(b) is emitted after A(b+1), so the short
        # latency-bound cross-engine softmax chain of batch b sits BEHIND
        # batch b+1's bulk mul/reduce work in every engine queue instead
        # of stalling it at each batch boundary.
        def emit_muls(b):
            subs = subs_all[b]
            sm32 = small_pool.tile([128, TCH], F32, name="sm32", tag="sm32",
                                   bufs=3)
            # products: ALL four subs as single fat [128,4,1024] DVE muls
            # (~780 ns/tile measured vs ~2860 ns/tile on the Pool)
            scrs = {}
            vbc = vreps[b][:].broadcast_to((128, NSUB, H))
            for s in range(NSUB):
                scrf = scr_pool.tile([128, NSUB, H], F16, name="scrf",
                                     tag="scrf", bufs=4)
                nc.vector.tensor_mul(scrf[:], subs[s][:], vbc)
                scrs[s] = scrf
            return sm32, scrs

        def emit_reduces(b, sm32, scrs):
            # 11 ACT accums / 5 DVE reduces per batch (measured balance)
            for k in range(TCH):
                s, j = k // NSUB, k % NSUB
                src = scrs[s][:, j, :]
                if j == 2 or k in (6, 13):
                    nc.vector.reduce_sum(sm32[:, k:k + 1], src,
                                         axis=mybir.AxisListType.X)
                else:
                    sink = scr_pool.tile([128, 1], F32, name="sink",
                                         tag="sink", bufs=6)
                    nc.scalar.activation(
                        sink.broadcast_to((128, H)), src,
                        mybir.ActivationFunctionType.Identity,
                        bias=0.0, scale=1.0,
                        accum_out=sm32[:, k:k + 1])

        ctx_sbs = []

        def emit_softmax(b, sm32):
            warm_pe(12)
            # additive mask bias (masked lanes -> -1e4)
            nc.vector.tensor_add(sm32[:], sm32[:], mb[:, b, :])
            # global max over [128, TCH] via PE transpose + bcast matmul
            colmax = small_pool.tile([128, 1], F32, name="colmax",
                                     tag="colmax", bufs=3)
            nc.vector.reduce_max(colmax[:], sm32[:], axis=mybir.AxisListType.X)
            spm = psum_small.tile([128, 160], F32, name="spm", tag="spm",
                                  bufs=2)
            nc.tensor.transpose(spm[:1, 0:128], colmax[:], ident[:])
            gmax = small_pool.tile([1, 1], F32, name="gmax", tag="gmax",
                                   bufs=2)
            nc.vector.reduce_max(gmax[:], spm[:1, 0:128],
                                 axis=mybir.AxisListType.X)
            nc.tensor.matmul(spm[:, 128:129], neg_row[:], gmax[:])
            negmax = small_pool.tile([128, 1], F32, name="negmax",
                                     tag="negmax", bufs=2)
            nc.vector.tensor_copy(negmax[:], spm[:, 128:129])
            # p = exp(score - max) fp16, exact fp32 row-sums via ACT accum
            emat = small_pool.tile([128, TCH], F16, name="emat", tag="emat",
                                   bufs=2)
            rowsum = small_pool.tile([128, 1], F32, name="rowsum",
                                     tag="rowsum", bufs=2)
            nc.scalar.activation(emat[:], sm32[:],
                                 mybir.ActivationFunctionType.Exp,
                                 bias=negmax[:], scale=1.0,
                                 accum_out=rowsum[:])
            # denom = sum_p rowsum[p]; rden = 1/denom (off the critical
            # path: runs while the context matmuls accumulate)
            nc.tensor.matmul(spm[:1, 129:130], rowsum[:], ones_col[:])
            rden = small_pool.tile([1, 1], F32, name="rden", tag="rden",
                                   bufs=2)
            nc.vector.reciprocal(rden[:], spm[:1, 129:130])
            return emat, rden

        def emit_ctx_mm(b, emat):
            # context[h] = sum_t exp[t] * enc[t, h] on the PE
            subs = subs_all[b]
            cps = psum_ctx.tile([1, H], F32, name="cps", tag="cps", bufs=2)
            for k in range(TCH):
                rhs = subs[k // NSUB]
                for hh in range(2):
                    nc.tensor.matmul(
                        cps[:, hh * 512:(hh + 1) * 512],
                        emat[:, k:k + 1],
                        rhs[:, k % NSUB, hh * 512:(hh + 1) * 512],
                        start=(k == 0), stop=(k == TCH - 1))
            return cps

        def emit_ctx_out(b, cps, rden):
            # normalized on the way out of PSUM by the Copy scale=1/denom
            ctx_sb = small_pool.tile([1, H], F32, name="ctx_sb", tag="ctx_sb",
                                     bufs=2)
            nc.scalar.activation(ctx_sb[:], cps[:],
                                 mybir.ActivationFunctionType.Copy,
                                 bias=0.0, scale=rden[:])
            ctx_sbs.append(ctx_sb)

        # Software-pipelined emission.  Batch b's latency-bound softmax
        # chain is slotted right after batch b+1's FIRST fat mul, so its
        # cross-engine hops resolve while bulk work drains; the PE context
        # matmuls are emitted before batch b+1's reduces, and the PSUM
        # evacuation copy after them (so the ACT never stalls on the PE).
        pend = None      # (b, sm32) awaiting softmax+ctx
        for b in range(B_LOC):
            sm32, scrs = emit_muls(b)
            emit_reduces(b, sm32, scrs)
            if pend is not None:
                emat_p, rden_p = emit_softmax(*pend)
                cps_p = emit_ctx_mm(pend[0], emat_p)
                emit_ctx_out(pend[0], cps_p, rden_p)
            pend = (b, sm32)
        emat_p, rden_p = emit_softmax(*pend)
        cps_p = emit_ctx_mm(pend[0], emat_p)
        emit_ctx_out(pend[0], cps_p, rden_p)

        # all output DMAs at the very end of the gpsimd (SWDGE) stream,
        # where they can't block any compute or enc trigger
        for b in range(B_LOC):
            nc.gpsimd.dma_start(out[b:b + 1, :], ctx_sbs[b][:])


def build_nc():
    """Build and compile the per-core Bass program."""
    nc = bacc.Bacc("TRN2", target_bir_lowering=False, debug=False,
                   enable_asserts=False, num_devices=N_CORES)
    enc_d = nc.dram_tensor("enc_hs", [B_LOC, T, H], F16, kind="ExternalInput")
    dect_d = nc.dram_tensor("dec_t", [128, 8, B_LOC], F16,
                            kind="ExternalInput")
    mbias_d = nc.dram_tensor("mbias", [128, B_LOC, TCH], F32,
                             kind="ExternalInput")
    wat_d = nc.dram_tensor("wat", [U, H], F16, kind="ExternalInput")
    out_d = nc.dram_tensor("context", [B_LOC, H], F32, kind="ExternalOutput")

    with tile.TileContext(nc) as tc:
        emit_kernel(tc, enc_d.ap(), dect_d.ap(), mbias_d.ap(), wat_d.ap(),
                    out_d.ap())
    nc.compile()
    return nc


def make_in_maps(enc_hs, dec_ht, mask, Wa):
    """Shard + lay out full inputs into per-core input maps."""
    enc16 = np.ascontiguousarray(np.asarray(enc_hs, dtype=np.float16))
    dec16 = np.asarray(dec_ht, dtype=np.float16)
    maskb = np.asarray(mask, dtype=bool)
    wat = np.ascontiguousarray(np.asarray(Wa, dtype=np.float16).T)
    in_maps = []
    for c in range(N_CORES):
        sl = slice(c * B_LOC, (c + 1) * B_LOC)
        # dec_t[p, ch, b] = dec[b, ch*128 + p]
        dec_t = np.ascontiguousarray(
            dec16[sl].T.reshape(8, 128, B_LOC).transpose(1, 0, 2))
        # mbias[p, b, k] = 0 where kept, -1e4 where masked (t = k*128 + p)
        mbias = np.ascontiguousarray(
            np.where(maskb[sl], np.float32(0.0), np.float32(MASK_BIAS))
            .astype(np.float32).reshape(B_LOC, TCH, 128).transpose(2, 0, 1))
        in_maps.append({
            "enc_hs": enc16[sl],
            "dec_t": dec_t,
            "mbias": mbias,
            "wat": wat,
        })
    return in_maps


_NC_CACHE = None


def get_nc():
    global _NC_CACHE
    if _NC_CACHE is None:
        _NC_CACHE = build_nc()
    return _NC_CACHE


def run_on_hw(enc_hs, dec_ht, mask, Wa, trace=False, **trace_kwargs):
    from concourse.bass_utils import run_bass_kernel_spmd
    nc = get_nc()
    in_maps = make_in_maps(enc_hs, dec_ht, mask, Wa)
    res = run_bass_kernel_spmd(nc, in_maps, list(range(N_CORES)), trace=trace,
                               **trace_kwargs)
    out = np.concatenate([res.results[c]["context"] for c in range(N_CORES)],
                         axis=0)
    return out.astype(np.float32), res


def kernel(enc_hs, dec_ht, mask, Wa):
    out, _ = run_on_hw(enc_hs, dec_ht, mask, Wa, trace=False)
    return out
